# revision 1
# baseline (speedup 1.0000x reference)
"""Expert-parallel MoE FFN for Trainium2 — one expert per NeuronCore (8 cores).

Strategy
--------
The reference computes, per token, the sum of top-2 expert FFN outputs (binary
combine mask, no gate weighting).  We shard along the expert axis: core ``e``
holds expert ``e``'s weights (W1[e], b1[e], W2[e], b2[e]) and processes only
the tokens that routed to it.

Host side (cheap, O(T*D*E) = 34 MFLOP):
  * gating softmax + top-2 (replicates jax.nn.softmax + jax.lax.top_k
    tie-breaking exactly: stable argsort on the fp32 scores, descending),
  * gather each expert's tokens, pad to a uniform capacity (all cores run the
    same NEFF), pre-transpose AND pre-pack every tensor into its exact SBUF
    layout ([128 partitions, flat free dim]) so each device DMA is a single
    trigger moving full-row (multi-KB) packets,
  * scatter-add the 8 per-expert outputs back into the [T, D] result.

Device side (the heavy part, ~18 GFLOP/core):
  hT = relu(W1^T-chained matmuls + b1);  yT = W2-chained matmuls + b2,
  everything kept in "transposed" layout: contraction dims live on SBUF
  partitions for both layers, so mm1's output feeds mm2 directly.
  bf16 inputs, fp32 PSUM accumulation.

DMA design: a trigger costs ~600 ns on its queue engine and sub-KB packets
halve DMA throughput, so every group is one trigger of contiguous rows.
Groups are ordered so the first token-tile's operands land first; W1/W2 ride
the GpSimd queue in parallel with x on the Sync queue.  Each group is its own
SBUF tensor, making Tile's whole-tile dependencies exact.
"""

import numpy as np
import ml_dtypes

import concourse.bacc as bacc
import concourse.mybir as mybir
import concourse.tile as tile
from concourse.bass_utils import run_bass_kernel_spmd
from concourse._compat import get_trn_type

D_MODEL = 1024
D_FF = 4096
N_EXP = 8
TOP_K = 2
KD = D_MODEL // 128  # 8 contraction chunks over d_model
KF = D_FF // 128  # 32 contraction chunks over d_ff

# W1 column groups, sized so each group lands ahead of mm1 consuming it;
# the first groups are small so mm1 can start early.
W1_GROUPS = [(0, 256), (256, 512), (512, 1536), (1536, 2560), (2560, D_FF)]
# W2 f-chunk groups (mm2 accumulates f ascending).
W2_GROUPS = [(0, KF // 2), (KF // 2, KF)]

BF16 = mybir.dt.bfloat16
F32 = mybir.dt.float32

_programs: dict[tuple, object] = {}


def _build_program(cap: int, tt: int):
    """Bass/Tile program: pre-packed [D,cap] tokens -> 2-layer FFN -> output."""
    assert cap % tt == 0
    nt = cap // tt
    nc = bacc.Bacc(get_trn_type() or "TRN2", target_bir_lowering=False, debug=False)

    # All inputs arrive pre-packed as [128, flat] in their SBUF layouts.
    # Token-tile 0 is split into two k-halves so the very first matmuls'
    # operands land first on the load ring.
    xg_names = ["x0a", "x0b"] + (["x1"] if nt > 1 else [])
    xg_widths = [KD // 2 * tt, KD // 2 * tt] + ([KD * (cap - tt)] if nt > 1 else [])
    xg_d = {
        n: nc.dram_tensor(n, [128, w], BF16, kind="ExternalInput").ap()
        for n, w in zip(xg_names, xg_widths)
    }
    w1_d = [
        nc.dram_tensor(f"W1{g}", [128, KD * (hi - lo)], BF16, kind="ExternalInput").ap()
        for g, (lo, hi) in enumerate(W1_GROUPS)
    ]
    w2_d = [
        nc.dram_tensor(f"W2{g}", [128, (fhi - flo) * D_MODEL], BF16,
                       kind="ExternalInput").ap()
        for g, (flo, fhi) in enumerate(W2_GROUPS)
    ]
    b1_d = nc.dram_tensor("b1", [128, KF], F32, kind="ExternalInput").ap()
    b2_d = nc.dram_tensor("b2", [128, KD], F32, kind="ExternalInput").ap()
    y_d = nc.dram_tensor("yT", [128, KD * cap], F32, kind="ExternalOutput").ap()
    y_v = y_d.rearrange("p (m c) -> p m c", c=cap)

    with tile.TileContext(nc) as tc:
        with (
            tc.tile_pool(name="sb", bufs=1) as sb,
            tc.tile_pool(name="hp", bufs=40) as hp,
            tc.tile_pool(name="yp", bufs=4) as yp,
            tc.tile_pool(name="pp1", bufs=6, space="PSUM") as pp1,
            tc.tile_pool(name="pp2", bufs=2, space="PSUM") as pp2,
        ):
            # ---- inputs --------------------------------------------------
            # All loads ride the single SWDGE ring serially, in exact
            # consumption order: one uncontended ring (~267 GB/s, prompt
            # completion sems) beats parallel rings — concurrent HWDGE
            # traffic starves the SWDGE ring that carries the operands the
            # first matmuls need (measured 267 -> ~50 GB/s).
            x_sb = {
                n: sb.tile([128, d.shape[1]], BF16, tag=n, name=f"{n}_sb")
                for n, d in xg_d.items()
            }
            w1_tiles = [
                sb.tile([128, KD * (hi - lo)], BF16, tag=f"w1g{g}", name=f"w1g{g}")
                for g, (lo, hi) in enumerate(W1_GROUPS)
            ]
            w1_gs = [(lo, hi, t) for (lo, hi), t in zip(W1_GROUPS, w1_tiles)]
            b1_sb = sb.tile([128, KF], F32, tag="b1", name="b1_sb")
            b2_sb = sb.tile([128, KD], F32, tag="b2", name="b2_sb")

            nc.gpsimd.dma_start(x_sb["x0a"][:], xg_d["x0a"])
            nc.gpsimd.dma_start(w1_tiles[0][:], w1_d[0])
            nc.gpsimd.dma_start(x_sb["x0b"][:], xg_d["x0b"])
            nc.gpsimd.dma_start(w1_tiles[1][:], w1_d[1])
            nc.gpsimd.dma_start(b1_sb[:], b1_d)
            nc.gpsimd.dma_start(b2_sb[:], b2_d)
            for g in range(2, len(W1_GROUPS)):
                nc.gpsimd.dma_start(w1_tiles[g][:], w1_d[g])

            def x_rhs(k, it):
                if it == 0:
                    ka = KD // 2
                    t = x_sb["x0a"] if k < ka else x_sb["x0b"]
                    kk = k if k < ka else k - ka
                    return t[:, kk * tt : (kk + 1) * tt]
                rest = cap - tt
                lo = k * rest + (it - 1) * tt
                return x_sb["x1"][:, lo : lo + tt]

            def w1_lhsT(k, f):
                col = f * 128
                for lo, hi, t in w1_gs:
                    if lo <= col < hi:
                        base = k * (hi - lo) + (col - lo)
                        return t[:, base : base + 128]
                raise AssertionError

            w2_gs = []
            for g, (flo, fhi) in enumerate(W2_GROUPS):
                t = sb.tile([128, (fhi - flo) * D_MODEL], BF16, tag=f"w2g{g}",
                            name=f"w2g{g}")
                nc.gpsimd.dma_start(t[:], w2_d[g])
                w2_gs.append((flo, fhi, t))

            if nt > 1:
                nc.gpsimd.dma_start(x_sb["x1"][:], xg_d["x1"])

            def w2_lhsT(f, m):
                for flo, fhi, t in w2_gs:
                    if flo <= f < fhi:
                        base = (f - flo) * D_MODEL + m * 128
                        return t[:, base : base + 128]
                raise AssertionError

            # ---- compute --------------------------------------------------
            for it in range(nt):
                # mm1: hT[f*128+p, t] = relu(sum_d W1[d, f*128+p]*xT[d, t] + b1)
                h_tiles = []
                for f in range(KF):
                    ps = pp1.tile([128, tt], F32, tag="ps1", name=f"ps1_{it}_{f}")
                    for k in range(KD):
                        nc.tensor.matmul(
                            ps[:],
                            w1_lhsT(k, f),
                            x_rhs(k, it),
                            start=(k == 0),
                            stop=(k == KD - 1),
                        )
                    ht = hp.tile([128, tt], BF16, tag="h", name=f"h_{it}_{f}")
                    nc.scalar.activation(
                        ht[:],
                        ps[:],
                        mybir.ActivationFunctionType.Relu,
                        bias=b1_sb[:, f : f + 1],
                    )
                    h_tiles.append(ht)

                # mm2: yT[m*128+p, t] = sum_f W2[f, m*128+p] * hT[f, t] + b2
                for m in range(KD):
                    ps2 = pp2.tile([128, tt], F32, tag="ps2", name=f"ps2_{it}_{m}")
                    for f in range(KF):
                        nc.tensor.matmul(
                            ps2[:],
                            w2_lhsT(f, m),
                            h_tiles[f][:],
                            start=(f == 0),
                            stop=(f == KF - 1),
                        )
                    yt = yp.tile([128, tt], F32, tag="y", name=f"y_{it}_{m}")
                    nc.vector.tensor_scalar_add(yt[:], ps2[:], b2_sb[:, m : m + 1])
                    nc.sync.dma_start(y_v[:, m, it * tt : (it + 1) * tt], yt[:])

    nc.compile()
    return nc


def _gating_topk(x, Wg, bg):
    """Replicates jax.nn.softmax + jax.lax.top_k(..., 2) in fp32 numpy."""
    logits = x @ Wg + bg
    m = logits.max(axis=1, keepdims=True)
    e = np.exp(logits - m)
    scores = e / e.sum(axis=1, keepdims=True)
    # top_k: descending, ties broken toward the lower index (stable).
    order = np.argsort(-scores, axis=1, kind="stable")
    return order[:, :TOP_K]


def _capacity(max_count):
    # Token tile <= 384: keeps one fp32 PSUM bank per matmul (<=512) AND the
    # resident-weights SBUF budget valid for capacities well beyond the
    # ~1024+-27 expert loads this distribution produces.
    nt = max(1, -(-max_count // 384))
    tt = -(-max_count // nt)
    tt = -(-tt // 4) * 4  # multiple of 4 for aligned fp32 rows
    return nt * tt, tt


def _pack_k128(a):
    """[K*128, F] -> [128, K*F]: partition-major packing of the SBUF layout."""
    k128, f = a.shape
    return np.ascontiguousarray(
        a.reshape(k128 // 128, 128, f).transpose(1, 0, 2).reshape(128, -1)
    )


def _prepare(x, Wg, bg, W1, b1, W2, b2):
    x = np.ascontiguousarray(np.asarray(x, dtype=np.float32))
    topk = _gating_topk(x, np.asarray(Wg, np.float32), np.asarray(bg, np.float32))
    idx = [np.nonzero((topk == e).any(axis=1))[0] for e in range(N_EXP)]
    counts = [len(i) for i in idx]
    cap, tt = _capacity(max(counts))
    nt = cap // tt

    bf16 = ml_dtypes.bfloat16
    in_maps = []
    for e in range(N_EXP):
        xg = np.zeros((cap, D_MODEL), np.float32)
        xg[: counts[e]] = x[idx[e]]
        xT = np.ascontiguousarray(xg.T).astype(bf16)  # [D, cap]
        xTp = _pack_k128(xT).reshape(128, KD, cap)  # [128, k, c]
        w1 = np.asarray(W1[e], np.float32).astype(bf16)  # [D, DFF]
        w1p = _pack_k128(w1).reshape(128, KD, D_FF)  # [128, k, f]
        w2 = np.asarray(W2[e], np.float32).astype(bf16)  # [DFF, D]
        w2p = _pack_k128(w2).reshape(128, KF, D_MODEL)  # [128, f, m]
        ka = KD // 2
        m = {
            "x0a": np.ascontiguousarray(xTp[:, :ka, :tt]).reshape(128, -1),
            "x0b": np.ascontiguousarray(xTp[:, ka:, :tt]).reshape(128, -1),
            "b1": np.ascontiguousarray(
                np.asarray(b1[e], np.float32).reshape(KF, 128).T
            ),
            "b2": np.ascontiguousarray(
                np.asarray(b2[e], np.float32).reshape(KD, 128).T
            ),
        }
        if nt > 1:
            m["x1"] = np.ascontiguousarray(xTp[:, :, tt:]).reshape(128, -1)
        for g, (lo, hi) in enumerate(W1_GROUPS):
            m[f"W1{g}"] = np.ascontiguousarray(w1p[:, :, lo:hi]).reshape(128, -1)
        for g, (flo, fhi) in enumerate(W2_GROUPS):
            m[f"W2{g}"] = np.ascontiguousarray(w2p[:, flo:fhi, :]).reshape(128, -1)
        in_maps.append(m)
    return x, idx, counts, cap, tt, in_maps


def _run(x, Wg, bg, W1, b1, W2, b2, **run_kwargs):
    x, idx, counts, cap, tt, in_maps = _prepare(x, Wg, bg, W1, b1, W2, b2)
    key = (cap, tt)
    prog = _programs.get(key)
    if prog is None:
        prog = _programs.setdefault(key, _build_program(cap, tt))
    res = run_bass_kernel_spmd(
        prog, in_maps, core_ids=list(range(N_EXP)), **run_kwargs
    )
    out = np.zeros_like(x)
    for e in range(N_EXP):
        yp = np.asarray(res.results[e]["yT"], np.float32)  # [128, KD*cap]
        yT = yp.reshape(128, KD, cap).transpose(1, 0, 2).reshape(D_MODEL, cap)
        out[idx[e]] += yT[:, : counts[e]].T
    return out, res


def kernel(x, Wg, bg, W1, b1, W2, b2):
    out, _ = _run(x, Wg, bg, W1, b1, W2, b2)
    return out



# revision 2
# speedup vs baseline: 1.0061x; 1.0061x over previous
"""Expert-parallel MoE FFN for Trainium2 — 4-way FF-split expert groups.

Strategy (v2)
-------------
The reference sums the top-2 experts' FFN outputs per token (binary mask, no
gate weighting).  v1 put one expert per core: the padded capacity is then the
MAX expert load (1091 for this distribution vs. 1024 mean), and every core
streams that many columns — ~6.5% wasted PE cycles.

v2 shards each expert's FF dimension 4 ways instead: experts are ranked by
token count and dealt into two groups of four (alternating, so the groups'
rank-j members have similar counts).  Core 4g+q holds quarter q (1024 of 4096
FF columns) of all four experts in group g:

    W1[e][:, q*1024:(q+1)*1024]  and  W2[e][q*1024:(q+1)*1024, :]

Each core runs all four of its group's expert segments back-to-back; the
4 quarter outputs of an expert are partial sums of its full FFN, added on the
host.  Per-slot capacity is the max of the TWO groups' rank-j counts, so the
total streamed columns drop from 8*max(c) to ~sum of per-rank maxes
(8736 -> ~8288 token-columns, -5%), with perfect static balance across cores.

Device kernel per core (shared NEFF; caps are per-rank, identical for both
groups):  for each segment: hT = relu(W1q^T-chained matmuls + b1q);
yT_partial = W2q-chained matmuls.  bf16 operands, fp32 PSUM.  b2 is added on
the host (out += mask @ b2), which also sums the 4 quarter partials.

Extras vs v1:
  * HAM warm-up: N_WARM dummy 512-col matmuls on a memset scratch tile keep
    the PE busy from boot (~8.4us) until the first real operands land
    (~19.5us), so the clock-gate (4/8 -> 8/8 after ~3.4us of activity) is
    released and never re-throttles — the old kernel ran its first ~7us at
    half clock and any mid-stream PE idle >3.4us repaid that penalty.
  * First DMA groups split small (256-token first tile, W1 f=0 chunk) so the
    first real matmul starts as early as the load ring allows.
  * y leaves the device in bf16, one large tile-major store per half-tile:
    per-(m,tile) fp32 stores backpressured the 8-matmul mm2 chains through
    the ps2/yt pools and stalled the PE (the 4-way split makes mm2 chains 4x
    shorter than v1's, so store frequency quadrupled).  Host sums the four
    quarter-partials in fp32; the extra bf16 rounding costs ~4e-4 rel err.

DMA design unchanged: serial SWDGE ring via gpsimd in exact consumption
order (one uncontended ring beats parallel rings), y stores on the sync
HWDGE queue.
"""

import numpy as np
import ml_dtypes

import concourse.bacc as bacc
import concourse.mybir as mybir
import concourse.tile as tile
from concourse.bass_utils import run_bass_kernel_spmd
from concourse._compat import get_trn_type

D_MODEL = 1024
D_FF = 4096
N_EXP = 8
TOP_K = 2
KD = D_MODEL // 128  # 8 contraction chunks over d_model
QF = D_FF // 4  # 1024 FF columns per quarter
KQ = QF // 128  # 8 chunks over the FF quarter
NS = 4  # expert slots per group
# HAM warm-up matmuls: cover boot (~8.4us) to first-operands-ready (~19.5us)
# so the PE never idles >3.4us early on (idle would re-throttle the clock
# gate to 4/8 and the first real tiles would run at half speed).
# 8 cold matmuls @ ~427ns release the gate, the rest run at ~305ns.
N_WARM = 34

BF16 = mybir.dt.bfloat16
F32 = mybir.dt.float32

_programs: dict[tuple, object] = {}


def _tiles(cap):
    """Evenly-sized token tiles of width <=512 covering cap columns.

    Even splitting (364/364/364 rather than 512/512/68) avoids tiny ragged
    tiles whose matmuls hit the ~60ns issue floor and whose y-store triggers
    (~0.6us each on Sync) outpace the compute.
    """
    nt = -(-cap // 512)
    tw = -(-cap // nt // 4) * 4
    out = []
    t0 = 0
    while t0 < cap:
        w = min(tw, cap - t0)
        out.append((t0, w))
        t0 += w
    return out


def _tiles0(cap):
    """Slot-0 tiling: a small 256-wide first tile, then even tiles.

    The first tile's operands gate the start of the real matmul stream; a
    256-token tile needs only ~0.8 MB off the load ring instead of ~1.3 MB.
    """
    assert cap > 768
    return [(0, 256)] + [(256 + t0, tw) for t0, tw in _tiles(cap - 256)]


def _build_program(caps: tuple):
    assert len(caps) == NS and caps[0] > 512
    nc = bacc.Bacc(get_trn_type() or "TRN2", target_bir_lowering=False, debug=False)

    cap0 = caps[0]
    tw0 = _tiles0(cap0)[0][1]  # slot-0 tile-0 width
    r0 = cap0 - tw0  # tile-1.. columns of slot 0
    xg_d = {
        "x0a": nc.dram_tensor("x0a", [128, 4 * tw0], BF16, kind="ExternalInput").ap(),
        "x0b": nc.dram_tensor("x0b", [128, 4 * tw0], BF16, kind="ExternalInput").ap(),
        "x0c": nc.dram_tensor("x0c", [128, KD * r0], BF16, kind="ExternalInput").ap(),
    }
    for s in range(1, NS):
        xg_d[f"x{s}"] = nc.dram_tensor(
            f"x{s}", [128, KD * caps[s]], BF16, kind="ExternalInput"
        ).ap()
    w1_d = {
        "w1_0a": nc.dram_tensor("w1_0a", [128, KD * 128], BF16,
                                kind="ExternalInput").ap(),
        "w1_0b": nc.dram_tensor("w1_0b", [128, (KQ - 1) * KD * 128], BF16,
                                kind="ExternalInput").ap(),
    }
    for s in range(1, NS):
        w1_d[f"w1_{s}"] = nc.dram_tensor(
            f"w1_{s}", [128, KQ * KD * 128], BF16, kind="ExternalInput"
        ).ap()
    w2_d = {
        "w2_0a": nc.dram_tensor("w2_0a", [128, (KD // 2) * KQ * 128], BF16,
                                kind="ExternalInput").ap(),
        "w2_0b": nc.dram_tensor("w2_0b", [128, (KD // 2) * KQ * 128], BF16,
                                kind="ExternalInput").ap(),
    }
    for s in range(1, NS):
        w2_d[f"w2_{s}"] = nc.dram_tensor(
            f"w2_{s}", [128, KD * KQ * 128], BF16, kind="ExternalInput"
        ).ap()
    b1_d = nc.dram_tensor("b1", [128, NS * KQ], F32, kind="ExternalInput").ap()
    # y is stored TILE-major in bf16: tile t's block is [128, KD*tw] at column
    # offset KD*t0 (m-major within the block).  One large contiguous store per
    # token tile keeps the HWDGE path (trigger ~0.6us + small-packet rate) off
    # the mm2 critical path — per-(m,tile) fp32 stores backpressured ps2/yt and
    # stalled the PE every chain.
    y_d = [
        nc.dram_tensor(f"y{s}", [128, KD * caps[s]], BF16, kind="ExternalOutput").ap()
        for s in range(NS)
    ]

    with tile.TileContext(nc) as tc:
        with (
            tc.tile_pool(name="sb", bufs=1) as sb,
            tc.tile_pool(name="xp", bufs=2) as xp,
            tc.tile_pool(name="hp", bufs=10) as hp,
            tc.tile_pool(name="yb", bufs=2) as yb,
            tc.tile_pool(name="pp1", bufs=6, space="PSUM") as pp1,
            tc.tile_pool(name="pp2", bufs=2, space="PSUM") as pp2,
        ):
            # ---- HAM warm-up: PE busy from boot so the clock-gate opens
            # (4/8 -> 8/8) before the first real operands land.  Reads a
            # memset scratch tile, writes a recycled PSUM bank.
            scratch = sb.tile([128, 512], BF16, tag="scr", name="scratch")
            nc.vector.memset(scratch[:], 0.0)
            wps = pp2.tile([128, 512], F32, tag="ps2", name="warm_ps")
            for i in range(N_WARM):
                nc.tensor.matmul(
                    wps[:], scratch[:, 0:128], scratch[:], start=True, stop=True
                )

            # ---- input tiles + loads in exact consumption order ----------
            x_sb = {
                n: (sb if n.startswith("x0") else xp).tile(
                    [128, d.shape[1]], BF16,
                    tag="xs" if not n.startswith("x0") else n,
                    name=f"{n}_sb",
                )
                for n, d in xg_d.items()
            }
            w1_sb = {
                n: sb.tile([128, d.shape[1]], BF16, tag=n, name=f"{n}_sb")
                for n, d in w1_d.items()
            }
            w2_sb = {
                n: sb.tile([128, d.shape[1]], BF16, tag=n, name=f"{n}_sb")
                for n, d in w2_d.items()
            }
            b1_sb = sb.tile([128, NS * KQ], F32, tag="b1", name="b1_sb")

            # b1 first (tiny, needed by the first relu); w2_0a before x0c so
            # slot-0's mm2 never catches the ring; x3 LAST — its transfer
            # stalls on a WAR hazard (it reuses x1's pool slot) and must not
            # block w1_3/w2_3 behind it on the serial ring.
            order = [
                "b1", "x0a", "w1_0a", "x0b", "w1_0b", "w2_0a", "x0c", "w2_0b",
                "x1", "w1_1", "w2_1", "x2", "w1_2", "w2_2",
                "w1_3", "w2_3", "x3",
            ]
            for n in order:
                if n.startswith("x"):
                    nc.gpsimd.dma_start(x_sb[n][:], xg_d[n])
                elif n.startswith("w1"):
                    nc.gpsimd.dma_start(w1_sb[n][:], w1_d[n])
                elif n.startswith("w2"):
                    nc.gpsimd.dma_start(w2_sb[n][:], w2_d[n])
                else:
                    nc.gpsimd.dma_start(b1_sb[:], b1_d)

            def x_rhs(s, k, t0, tw):
                if s == 0:
                    if t0 == 0:
                        t = x_sb["x0a"] if k < 4 else x_sb["x0b"]
                        kk = k if k < 4 else k - 4
                        return t[:, kk * tw0 : kk * tw0 + tw]
                    lo = k * r0 + (t0 - tw0)
                    return x_sb["x0c"][:, lo : lo + tw]
                lo = k * caps[s] + t0
                return x_sb[f"x{s}"][:, lo : lo + tw]

            def w1_lhsT(s, f, k):
                if s == 0:
                    if f == 0:
                        return w1_sb["w1_0a"][:, k * 128 : (k + 1) * 128]
                    base = ((f - 1) * KD + k) * 128
                    return w1_sb["w1_0b"][:, base : base + 128]
                base = (f * KD + k) * 128
                return w1_sb[f"w1_{s}"][:, base : base + 128]

            def w2_lhsT(s, m, f):
                if s == 0:
                    t = w2_sb["w2_0a"] if m < KD // 2 else w2_sb["w2_0b"]
                    mm = m if m < KD // 2 else m - KD // 2
                    return t[:, (mm * KQ + f) * 128 : (mm * KQ + f) * 128 + 128]
                base = (m * KQ + f) * 128
                return w2_sb[f"w2_{s}"][:, base : base + 128]

            # ---- compute --------------------------------------------------
            for s in range(NS):
                s_tiles = _tiles0(caps[s]) if s == 0 else _tiles(caps[s])
                for it, (t0, tw) in enumerate(s_tiles):
                    h_tiles = []
                    for f in range(KQ):
                        ps = pp1.tile([128, tw], F32, tag="ps1",
                                      name=f"ps1_{s}_{it}_{f}")
                        for k in range(KD):
                            nc.tensor.matmul(
                                ps[:],
                                w1_lhsT(s, f, k),
                                x_rhs(s, k, t0, tw),
                                start=(k == 0),
                                stop=(k == KD - 1),
                            )
                        ht = hp.tile([128, tw], BF16, tag="h",
                                     name=f"h_{s}_{it}_{f}")
                        nc.scalar.activation(
                            ht[:],
                            ps[:],
                            mybir.ActivationFunctionType.Relu,
                            bias=b1_sb[:, s * KQ + f : s * KQ + f + 1],
                        )
                        h_tiles.append(ht)

                    ybt = yb.tile([128, KD * tw], BF16, tag="y",
                                  name=f"y_{s}_{it}")
                    for m in range(KD):
                        ps2 = pp2.tile([128, tw], F32, tag="ps2",
                                       name=f"ps2_{s}_{it}_{m}")
                        for f in range(KQ):
                            nc.tensor.matmul(
                                ps2[:],
                                w2_lhsT(s, m, f),
                                h_tiles[f][:],
                                start=(f == 0),
                                stop=(f == KQ - 1),
                            )
                        nc.vector.tensor_scalar_add(
                            ybt[:, m * tw : (m + 1) * tw], ps2[:], 0.0
                        )
                        # store in halves so the final store (kernel tail)
                        # is small and the first half overlaps chains 4..7
                        if m == KD // 2 - 1:
                            nc.sync.dma_start(
                                y_d[s][:, KD * t0 : KD * t0 + 4 * tw],
                                ybt[:, : 4 * tw],
                            )
                        elif m == KD - 1:
                            nc.sync.dma_start(
                                y_d[s][:, KD * t0 + 4 * tw : KD * (t0 + tw)],
                                ybt[:, 4 * tw :],
                            )

    nc.compile()
    return nc


def _gating_topk(x, Wg, bg):
    """Replicates jax.nn.softmax + jax.lax.top_k(..., 2) in fp32 numpy."""
    logits = x @ Wg + bg
    m = logits.max(axis=1, keepdims=True)
    e = np.exp(logits - m)
    scores = e / e.sum(axis=1, keepdims=True)
    order = np.argsort(-scores, axis=1, kind="stable")
    return order[:, :TOP_K]


def _pack_k128(a):
    """[K*128, F] -> [128, K*F]: partition-major packing of the SBUF layout."""
    k128, f = a.shape
    return np.ascontiguousarray(
        a.reshape(k128 // 128, 128, f).transpose(1, 0, 2).reshape(128, -1)
    )


def _prepare(x, Wg, bg, W1, b1, W2, b2):
    x = np.ascontiguousarray(np.asarray(x, dtype=np.float32))
    topk = _gating_topk(x, np.asarray(Wg, np.float32), np.asarray(bg, np.float32))
    idx = [np.nonzero((topk == e).any(axis=1))[0] for e in range(N_EXP)]
    counts = np.array([len(i) for i in idx])

    # Rank experts by load; deal into two groups of four so rank-j members
    # have similar counts; per-rank capacity = max over the two groups.
    rank = np.argsort(-counts, kind="stable")
    groups = [rank[0::2], rank[1::2]]
    caps = tuple(
        int(-(-max(counts[groups[0][j]], counts[groups[1][j]]) // 4) * 4)
        for j in range(NS)
    )

    bf16 = ml_dtypes.bfloat16
    in_maps = [dict() for _ in range(2 * NS)]
    for g in range(2):
        for s in range(NS):
            e = int(groups[g][s])
            cap = caps[s]
            xg = np.zeros((cap, D_MODEL), np.float32)
            xg[: counts[e]] = x[idx[e]]
            xT = np.ascontiguousarray(xg.T).astype(bf16)  # [D, cap]
            xTp = _pack_k128(xT).reshape(128, KD, cap)  # [128, k, c]
            if s == 0:
                tw0 = _tiles0(cap)[0][1]
                xa = np.ascontiguousarray(xTp[:, 0:4, 0:tw0]).reshape(128, -1)
                xb = np.ascontiguousarray(xTp[:, 4:8, 0:tw0]).reshape(128, -1)
                xc = np.ascontiguousarray(xTp[:, :, tw0:]).reshape(128, -1)
            w1e = np.asarray(W1[e], np.float32).astype(bf16)  # [D, DFF]
            w2e = np.asarray(W2[e], np.float32).astype(bf16)  # [DFF, D]
            b1e = np.asarray(b1[e], np.float32)
            for q in range(4):
                m = in_maps[4 * g + q]
                w1q = w1e[:, q * QF : (q + 1) * QF]  # [1024, 1024]
                # chunk (f,k): [128(d) x 128(fcol)] at offset (f*KD+k)*128
                w1p = np.ascontiguousarray(
                    w1q.reshape(KD, 128, KQ, 128).transpose(1, 2, 0, 3)
                ).reshape(128, -1)
                w2q = w2e[q * QF : (q + 1) * QF, :]  # [1024, 1024]
                # chunk (m,f): [128(ff) x 128(mcol)] at offset (m*KQ+f)*128
                w2p = np.ascontiguousarray(
                    w2q.reshape(KQ, 128, KD, 128).transpose(1, 2, 0, 3)
                ).reshape(128, -1)
                if s == 0:
                    m["x0a"], m["x0b"], m["x0c"] = xa, xb, xc
                    m["w1_0a"] = np.ascontiguousarray(w1p[:, : KD * 128])
                    m["w1_0b"] = np.ascontiguousarray(w1p[:, KD * 128 :])
                    half = (KD // 2) * KQ * 128
                    m["w2_0a"] = np.ascontiguousarray(w2p[:, :half])
                    m["w2_0b"] = np.ascontiguousarray(w2p[:, half:])
                else:
                    m[f"x{s}"] = np.ascontiguousarray(xTp).reshape(128, -1)
                    m[f"w1_{s}"] = w1p
                    m[f"w2_{s}"] = w2p
                # b1 packed: col s*KQ+f, row p = b1[e][q*QF + f*128 + p]
                b1q = b1e[q * QF : (q + 1) * QF].reshape(KQ, 128).T  # [128, KQ]
                if "b1" not in m:
                    m["b1"] = np.zeros((128, NS * KQ), np.float32)
                m["b1"][:, s * KQ : (s + 1) * KQ] = b1q
    for m in in_maps:
        m["b1"] = np.ascontiguousarray(m["b1"])

    mask = np.zeros((x.shape[0], N_EXP), np.float32)
    np.put_along_axis(mask, topk, 1.0, axis=1)
    return x, idx, counts, caps, groups, mask, in_maps


def _run(x, Wg, bg, W1, b1, W2, b2, **run_kwargs):
    x, idx, counts, caps, groups, mask, in_maps = _prepare(
        x, Wg, bg, W1, b1, W2, b2
    )
    prog = _programs.get(caps)
    if prog is None:
        prog = _programs.setdefault(caps, _build_program(caps))
    res = run_bass_kernel_spmd(
        prog, in_maps, core_ids=list(range(2 * NS)), **run_kwargs
    )
    out = np.zeros_like(x)
    for g in range(2):
        for s in range(NS):
            e = int(groups[g][s])
            cap = caps[s]
            acc = np.zeros((128, KD * cap), np.float32)
            for q in range(4):
                acc += np.asarray(res.results[4 * g + q][f"y{s}"]).astype(
                    np.float32
                )
            # tile-major: block t = [:, KD*t0 : KD*(t0+tw)] holds [p, m, tw]
            yT = np.empty((D_MODEL, cap), np.float32)
            for t0, tw in (_tiles0(cap) if s == 0 else _tiles(cap)):
                blk = acc[:, KD * t0 : KD * (t0 + tw)].reshape(128, KD, tw)
                yT[:, t0 : t0 + tw] = blk.transpose(1, 0, 2).reshape(
                    D_MODEL, tw
                )
            out[idx[e]] += yT[:, : counts[e]].T
    out += mask @ np.asarray(b2, np.float32)
    return out, res


def kernel(x, Wg, bg, W1, b1, W2, b2):
    out, _ = _run(x, Wg, bg, W1, b1, W2, b2)
    return out


# revision 3
# speedup vs baseline: 1.0208x; 1.0146x over previous
"""Expert-parallel MoE FFN for Trainium2 — 4-way FF-split expert groups.

Strategy (v2)
-------------
The reference sums the top-2 experts' FFN outputs per token (binary mask, no
gate weighting).  v1 put one expert per core: the padded capacity is then the
MAX expert load (1091 for this distribution vs. 1024 mean), and every core
streams that many columns — ~6.5% wasted PE cycles.

v2 shards each expert's FF dimension 4 ways instead: experts are ranked by
token count and dealt into two groups of four (alternating, so the groups'
rank-j members have similar counts).  Core 4g+q holds quarter q (1024 of 4096
FF columns) of all four experts in group g:

    W1[e][:, q*1024:(q+1)*1024]  and  W2[e][q*1024:(q+1)*1024, :]

Each core runs all four of its group's expert segments back-to-back; the
4 quarter outputs of an expert are partial sums of its full FFN, added on the
host.  Per-slot capacity is the max of the TWO groups' rank-j counts, so the
total streamed columns drop from 8*max(c) to ~sum of per-rank maxes
(8736 -> ~8288 token-columns, -5%), with perfect static balance across cores.

Device kernel per core (shared NEFF; caps are per-rank, identical for both
groups):  for each segment: hT = relu(W1q^T-chained matmuls + b1q);
yT_partial = W2q-chained matmuls.  bf16 operands, fp32 PSUM.  b2 is added on
the host (out += mask @ b2), which also sums the 4 quarter partials.

Extras vs v1:
  * HAM warm-up: N_WARM dummy 512-col matmuls on a memset scratch tile keep
    the PE busy from boot (~8.4us) until the first real operands land
    (~19.5us), so the clock-gate (4/8 -> 8/8 after ~3.4us of activity) is
    released and never re-throttles — the old kernel ran its first ~7us at
    half clock and any mid-stream PE idle >3.4us repaid that penalty.
  * First DMA groups split small (256-token first tile, W1 f=0 chunk) so the
    first real matmul starts as early as the load ring allows.
  * y leaves the device in bf16, one large tile-major store per half-tile:
    per-(m,tile) fp32 stores backpressured the 8-matmul mm2 chains through
    the ps2/yt pools and stalled the PE (the 4-way split makes mm2 chains 4x
    shorter than v1's, so store frequency quadrupled).  Host sums the four
    quarter-partials in fp32; the extra bf16 rounding costs ~4e-4 rel err.

DMA design unchanged: serial SWDGE ring via gpsimd in exact consumption
order (one uncontended ring beats parallel rings), y stores on the sync
HWDGE queue.
"""

import numpy as np
import ml_dtypes

import concourse.bacc as bacc
import concourse.mybir as mybir
import concourse.tile as tile
from concourse.bass_utils import run_bass_kernel_spmd
from concourse._compat import get_trn_type

D_MODEL = 1024
D_FF = 4096
N_EXP = 8
TOP_K = 2
KD = D_MODEL // 128  # 8 contraction chunks over d_model
QF = D_FF // 4  # 1024 FF columns per quarter
KQ = QF // 128  # 8 chunks over the FF quarter
NS = 4  # expert slots per group
# HAM warm-up matmuls: cover boot (~8.4us) to first-operands-ready (~19.5us)
# so the PE never idles >3.4us early on (idle would re-throttle the clock
# gate to 4/8 and the first real tiles would run at half speed).
# 8 cold matmuls @ ~427ns release the gate, the rest run at ~305ns.
N_WARM = 30

BF16 = mybir.dt.bfloat16
F32 = mybir.dt.float32

_programs: dict[tuple, object] = {}


def _tiles(cap):
    """Evenly-sized token tiles of width <=512 covering cap columns.

    Even splitting (364/364/364 rather than 512/512/68) avoids tiny ragged
    tiles whose matmuls hit the ~60ns issue floor and whose y-store triggers
    (~0.6us each on Sync) outpace the compute.
    """
    nt = -(-cap // 512)
    tw = -(-cap // nt // 4) * 4
    out = []
    t0 = 0
    while t0 < cap:
        w = min(tw, cap - t0)
        out.append((t0, w))
        t0 += w
    return out


def _tiles0(cap):
    """Slot-0 tiling: a small 256-wide first tile, then even tiles.

    The first tile's operands gate the start of the real matmul stream; a
    256-token tile needs only ~0.8 MB off the load ring instead of ~1.3 MB.
    """
    assert cap > 768
    return [(0, 256)] + [(256 + t0, tw) for t0, tw in _tiles(cap - 256)]


def _build_program(caps: tuple):
    assert len(caps) == NS and caps[0] > 512
    nc = bacc.Bacc(get_trn_type() or "TRN2", target_bir_lowering=False, debug=False)

    cap0 = caps[0]
    tw0 = _tiles0(cap0)[0][1]  # slot-0 tile-0 width
    r0 = cap0 - tw0  # tile-1.. columns of slot 0
    xg_d = {
        "x0ab": nc.dram_tensor("x0ab", [128, KD * tw0], BF16,
                               kind="ExternalInput").ap(),
        "x0c": nc.dram_tensor("x0c", [128, KD * r0], BF16, kind="ExternalInput").ap(),
    }
    for s in range(1, NS):
        xg_d[f"x{s}"] = nc.dram_tensor(
            f"x{s}", [128, KD * caps[s]], BF16, kind="ExternalInput"
        ).ap()
    w1_d = {
        "w1_0a": nc.dram_tensor("w1_0a", [128, KD * 128], BF16,
                                kind="ExternalInput").ap(),
        "w1_0b": nc.dram_tensor("w1_0b", [128, (KQ - 1) * KD * 128], BF16,
                                kind="ExternalInput").ap(),
    }
    for s in range(1, NS):
        w1_d[f"w1_{s}"] = nc.dram_tensor(
            f"w1_{s}", [128, KQ * KD * 128], BF16, kind="ExternalInput"
        ).ap()
    w2_d = {
        "w2_0a": nc.dram_tensor("w2_0a", [128, (KD // 2) * KQ * 128], BF16,
                                kind="ExternalInput").ap(),
        "w2_0b": nc.dram_tensor("w2_0b", [128, (KD // 2) * KQ * 128], BF16,
                                kind="ExternalInput").ap(),
    }
    for s in range(1, NS):
        w2_d[f"w2_{s}"] = nc.dram_tensor(
            f"w2_{s}", [128, KD * KQ * 128], BF16, kind="ExternalInput"
        ).ap()
    b1_d = nc.dram_tensor("b1", [128, NS * KQ], F32, kind="ExternalInput").ap()
    # y is stored TILE-major in bf16: tile t's block is [128, KD*tw] at column
    # offset KD*t0 (m-major within the block).  One large contiguous store per
    # token tile keeps the HWDGE path (trigger ~0.6us + small-packet rate) off
    # the mm2 critical path — per-(m,tile) fp32 stores backpressured ps2/yt and
    # stalled the PE every chain.
    y_d = [
        nc.dram_tensor(f"y{s}", [128, KD * caps[s]], BF16, kind="ExternalOutput").ap()
        for s in range(NS)
    ]

    with tile.TileContext(nc) as tc:
        with (
            tc.tile_pool(name="sb", bufs=1) as sb,
            tc.tile_pool(name="xp", bufs=2) as xp,
            tc.tile_pool(name="hp", bufs=10) as hp,
            tc.tile_pool(name="yb", bufs=2) as yb,
            tc.tile_pool(name="pp1", bufs=6, space="PSUM") as pp1,
            tc.tile_pool(name="pp2", bufs=2, space="PSUM") as pp2,
        ):
            # ---- HAM warm-up: PE busy from boot so the clock-gate opens
            # (4/8 -> 8/8) before the first real operands land.  Reads a
            # memset scratch tile, writes a recycled PSUM bank.
            scratch = sb.tile([128, 512], BF16, tag="scr", name="scratch")
            nc.vector.memset(scratch[:], 0.0)
            wps = pp2.tile([128, 512], F32, tag="ps2", name="warm_ps")
            for i in range(N_WARM):
                nc.tensor.matmul(
                    wps[:], scratch[:, 0:128], scratch[:], start=True, stop=True
                )

            # ---- input tiles + loads in exact consumption order ----------
            x_sb = {
                n: (sb if n.startswith("x0") else xp).tile(
                    [128, d.shape[1]], BF16,
                    tag="xs" if not n.startswith("x0") else n,
                    name=f"{n}_sb",
                )
                for n, d in xg_d.items()
            }
            w1_sb = {
                n: sb.tile([128, d.shape[1]], BF16, tag=n, name=f"{n}_sb")
                for n, d in w1_d.items()
            }
            w2_sb = {
                n: sb.tile([128, d.shape[1]], BF16, tag=n, name=f"{n}_sb")
                for n, d in w2_d.items()
            }
            b1_sb = sb.tile([128, NS * KQ], F32, tag="b1", name="b1_sb")

            # b1 first (tiny, needed by the first relu); w2_0a before x0c so
            # slot-0's mm2 never catches the ring; x3 LAST — its transfer
            # stalls on a WAR hazard (it reuses x1's pool slot) and must not
            # block w1_3/w2_3 behind it on the serial ring.
            order = [
                "x0ab", "w1_0a", "b1", "w1_0b", "w2_0a", "x0c", "w2_0b",
                "x1", "w1_1", "w2_1", "x2", "w1_2", "w2_2",
                "w1_3", "w2_3", "x3",
            ]
            # The first real matmuls are gated on x0ab + w1_0a; ship those on
            # the sync HWDGE queue, which sits idle until the first y store
            # (~40us), in parallel with the main gpsimd SWDGE ring.  The
            # SWDGE-starvation concern only bites under sustained concurrent
            # HWDGE traffic; this is 0.8 MB once at boot.
            for n in order:
                eng = nc.sync if n in ("x0ab", "w1_0a") else nc.gpsimd
                if n.startswith("x"):
                    eng.dma_start(x_sb[n][:], xg_d[n])
                elif n.startswith("w1"):
                    eng.dma_start(w1_sb[n][:], w1_d[n])
                elif n.startswith("w2"):
                    eng.dma_start(w2_sb[n][:], w2_d[n])
                else:
                    eng.dma_start(b1_sb[:], b1_d)

            def x_rhs(s, k, t0, tw):
                if s == 0:
                    if t0 == 0:
                        return x_sb["x0ab"][:, k * tw0 : k * tw0 + tw]
                    lo = k * r0 + (t0 - tw0)
                    return x_sb["x0c"][:, lo : lo + tw]
                lo = k * caps[s] + t0
                return x_sb[f"x{s}"][:, lo : lo + tw]

            def w1_lhsT(s, f, k):
                if s == 0:
                    if f == 0:
                        return w1_sb["w1_0a"][:, k * 128 : (k + 1) * 128]
                    base = ((f - 1) * KD + k) * 128
                    return w1_sb["w1_0b"][:, base : base + 128]
                base = (f * KD + k) * 128
                return w1_sb[f"w1_{s}"][:, base : base + 128]

            def w2_lhsT(s, m, f):
                if s == 0:
                    t = w2_sb["w2_0a"] if m < KD // 2 else w2_sb["w2_0b"]
                    mm = m if m < KD // 2 else m - KD // 2
                    return t[:, (mm * KQ + f) * 128 : (mm * KQ + f) * 128 + 128]
                base = (m * KQ + f) * 128
                return w2_sb[f"w2_{s}"][:, base : base + 128]

            # ---- compute --------------------------------------------------
            for s in range(NS):
                s_tiles = _tiles0(caps[s]) if s == 0 else _tiles(caps[s])
                for it, (t0, tw) in enumerate(s_tiles):
                    h_tiles = []
                    for f in range(KQ):
                        ps = pp1.tile([128, tw], F32, tag="ps1",
                                      name=f"ps1_{s}_{it}_{f}")
                        for k in range(KD):
                            nc.tensor.matmul(
                                ps[:],
                                w1_lhsT(s, f, k),
                                x_rhs(s, k, t0, tw),
                                start=(k == 0),
                                stop=(k == KD - 1),
                            )
                        ht = hp.tile([128, tw], BF16, tag="h",
                                     name=f"h_{s}_{it}_{f}")
                        nc.scalar.activation(
                            ht[:],
                            ps[:],
                            mybir.ActivationFunctionType.Relu,
                            bias=b1_sb[:, s * KQ + f : s * KQ + f + 1],
                        )
                        h_tiles.append(ht)

                    ybt = yb.tile([128, KD * tw], BF16, tag="y",
                                  name=f"y_{s}_{it}")
                    for m in range(KD):
                        ps2 = pp2.tile([128, tw], F32, tag="ps2",
                                       name=f"ps2_{s}_{it}_{m}")
                        for f in range(KQ):
                            nc.tensor.matmul(
                                ps2[:],
                                w2_lhsT(s, m, f),
                                h_tiles[f][:],
                                start=(f == 0),
                                stop=(f == KQ - 1),
                            )
                        nc.vector.tensor_scalar_add(
                            ybt[:, m * tw : (m + 1) * tw], ps2[:], 0.0
                        )
                        # store in halves so the final store (kernel tail)
                        # is small and the first half overlaps chains 4..7
                        if m == KD // 2 - 1:
                            nc.sync.dma_start(
                                y_d[s][:, KD * t0 : KD * t0 + 4 * tw],
                                ybt[:, : 4 * tw],
                            )
                        elif m == KD - 1:
                            nc.sync.dma_start(
                                y_d[s][:, KD * t0 + 4 * tw : KD * (t0 + tw)],
                                ybt[:, 4 * tw :],
                            )

    nc.compile()
    return nc


def _gating_topk(x, Wg, bg):
    """Replicates jax.nn.softmax + jax.lax.top_k(..., 2) in fp32 numpy."""
    logits = x @ Wg + bg
    m = logits.max(axis=1, keepdims=True)
    e = np.exp(logits - m)
    scores = e / e.sum(axis=1, keepdims=True)
    order = np.argsort(-scores, axis=1, kind="stable")
    return order[:, :TOP_K]


def _pack_k128(a):
    """[K*128, F] -> [128, K*F]: partition-major packing of the SBUF layout."""
    k128, f = a.shape
    return np.ascontiguousarray(
        a.reshape(k128 // 128, 128, f).transpose(1, 0, 2).reshape(128, -1)
    )


def _prepare(x, Wg, bg, W1, b1, W2, b2):
    x = np.ascontiguousarray(np.asarray(x, dtype=np.float32))
    topk = _gating_topk(x, np.asarray(Wg, np.float32), np.asarray(bg, np.float32))
    idx = [np.nonzero((topk == e).any(axis=1))[0] for e in range(N_EXP)]
    counts = np.array([len(i) for i in idx])

    # Rank experts by load; deal into two groups of four so rank-j members
    # have similar counts; per-rank capacity = max over the two groups.
    rank = np.argsort(-counts, kind="stable")
    groups = [rank[0::2], rank[1::2]]
    caps = tuple(
        int(-(-max(counts[groups[0][j]], counts[groups[1][j]]) // 4) * 4)
        for j in range(NS)
    )

    bf16 = ml_dtypes.bfloat16
    in_maps = [dict() for _ in range(2 * NS)]
    for g in range(2):
        for s in range(NS):
            e = int(groups[g][s])
            cap = caps[s]
            xg = np.zeros((cap, D_MODEL), np.float32)
            xg[: counts[e]] = x[idx[e]]
            xT = np.ascontiguousarray(xg.T).astype(bf16)  # [D, cap]
            xTp = _pack_k128(xT).reshape(128, KD, cap)  # [128, k, c]
            if s == 0:
                tw0 = _tiles0(cap)[0][1]
                xab = np.ascontiguousarray(xTp[:, :, 0:tw0]).reshape(128, -1)
                xc = np.ascontiguousarray(xTp[:, :, tw0:]).reshape(128, -1)
            w1e = np.asarray(W1[e], np.float32).astype(bf16)  # [D, DFF]
            w2e = np.asarray(W2[e], np.float32).astype(bf16)  # [DFF, D]
            b1e = np.asarray(b1[e], np.float32)
            for q in range(4):
                m = in_maps[4 * g + q]
                w1q = w1e[:, q * QF : (q + 1) * QF]  # [1024, 1024]
                # chunk (f,k): [128(d) x 128(fcol)] at offset (f*KD+k)*128
                w1p = np.ascontiguousarray(
                    w1q.reshape(KD, 128, KQ, 128).transpose(1, 2, 0, 3)
                ).reshape(128, -1)
                w2q = w2e[q * QF : (q + 1) * QF, :]  # [1024, 1024]
                # chunk (m,f): [128(ff) x 128(mcol)] at offset (m*KQ+f)*128
                w2p = np.ascontiguousarray(
                    w2q.reshape(KQ, 128, KD, 128).transpose(1, 2, 0, 3)
                ).reshape(128, -1)
                if s == 0:
                    m["x0ab"], m["x0c"] = xab, xc
                    m["w1_0a"] = np.ascontiguousarray(w1p[:, : KD * 128])
                    m["w1_0b"] = np.ascontiguousarray(w1p[:, KD * 128 :])
                    half = (KD // 2) * KQ * 128
                    m["w2_0a"] = np.ascontiguousarray(w2p[:, :half])
                    m["w2_0b"] = np.ascontiguousarray(w2p[:, half:])
                else:
                    m[f"x{s}"] = np.ascontiguousarray(xTp).reshape(128, -1)
                    m[f"w1_{s}"] = w1p
                    m[f"w2_{s}"] = w2p
                # b1 packed: col s*KQ+f, row p = b1[e][q*QF + f*128 + p]
                b1q = b1e[q * QF : (q + 1) * QF].reshape(KQ, 128).T  # [128, KQ]
                if "b1" not in m:
                    m["b1"] = np.zeros((128, NS * KQ), np.float32)
                m["b1"][:, s * KQ : (s + 1) * KQ] = b1q
    for m in in_maps:
        m["b1"] = np.ascontiguousarray(m["b1"])

    mask = np.zeros((x.shape[0], N_EXP), np.float32)
    np.put_along_axis(mask, topk, 1.0, axis=1)
    return x, idx, counts, caps, groups, mask, in_maps


def _run(x, Wg, bg, W1, b1, W2, b2, **run_kwargs):
    x, idx, counts, caps, groups, mask, in_maps = _prepare(
        x, Wg, bg, W1, b1, W2, b2
    )
    prog = _programs.get(caps)
    if prog is None:
        prog = _programs.setdefault(caps, _build_program(caps))
    res = run_bass_kernel_spmd(
        prog, in_maps, core_ids=list(range(2 * NS)), **run_kwargs
    )
    out = np.zeros_like(x)
    for g in range(2):
        for s in range(NS):
            e = int(groups[g][s])
            cap = caps[s]
            acc = np.zeros((128, KD * cap), np.float32)
            for q in range(4):
                acc += np.asarray(res.results[4 * g + q][f"y{s}"]).astype(
                    np.float32
                )
            # tile-major: block t = [:, KD*t0 : KD*(t0+tw)] holds [p, m, tw]
            yT = np.empty((D_MODEL, cap), np.float32)
            for t0, tw in (_tiles0(cap) if s == 0 else _tiles(cap)):
                blk = acc[:, KD * t0 : KD * (t0 + tw)].reshape(128, KD, tw)
                yT[:, t0 : t0 + tw] = blk.transpose(1, 0, 2).reshape(
                    D_MODEL, tw
                )
            out[idx[e]] += yT[:, : counts[e]].T
    out += mask @ np.asarray(b2, np.float32)
    return out, res


def kernel(x, Wg, bg, W1, b1, W2, b2):
    out, _ = _run(x, Wg, bg, W1, b1, W2, b2)
    return out
